# revision 19
# baseline (speedup 1.0000x reference)
"""Bahdanau attention on Trainium2, data-parallel over batch across 8 NeuronCores.

reference math (per batch b):
  q_proj = query @ Wa_w.T + Wa_b            # [1, H]
  k_proj = keys  @ Ua_w.T + Ua_b            # [S, H]
  e      = tanh(q_proj + k_proj)            # [S, H]
  scores = e @ Va_w[0] + Va_b[0]            # [S]
  weights= softmax(scores)                  # [S]
  context= weights @ keys                   # [H]

Device strategy (per core, B_loc = 4 batches):
  - keys tiles [128s, 1024h] stay resident in SBUF for the whole batch; they
    serve both the k_proj matmul (after a PE transpose to [h, s] layout) and
    the context matmul (natural layout, contraction over s).
  - k_proj is computed transposed: kprojT[k, s] = UaT.T @ keysT, with UaT
    (the [h, k]-layout Ua) resident in SBUF; fp32r matmuls at full PE rate.
  - tanh folds in the per-k bias (q_proj[k] + Wa_b[k] + Ua_b[k]) via the
    ACT per-partition bias operand; Va dot-product is a PE matvec over k.
  - softmax skips the max-subtraction (|scores| <= ||Va||_1 ~ 16, exp is
    safe in fp32; softmax is shift-invariant so Va_b drops out entirely).
    The denominator comes free from the ACT accum_out of the exp.
"""

import numpy as np

import concourse.bass as bass
import concourse.tile as tile
from concourse import bacc, mybir
from concourse.bass_utils import run_bass_kernel_spmd

F32 = mybir.dt.float32
F32R = mybir.dt.float32r

B, S, H = 32, 2048, 1024
N_CORES = 8
B_LOC = B // N_CORES  # 4


def build_bass(b_loc=B_LOC, s=S, h=H):
    ht = h // 128    # h-tiles (contraction tiles)
    kb_n = h // 128  # k-blocks (output tiles of k_proj)
    ssb = 512        # s-superblock: matmul moving-dim size
    n_ssb = s // ssb
    n_st = s // 128  # s-tiles per batch
    n_hh = h // 512  # 512-wide column halves for the context matmul

    nc = bacc.Bacc("TRN2", target_bir_lowering=False, debug=False)

    keys_d = nc.declare_dram_parameter("keys", [b_loc, s, h], F32R, isOutput=False)
    uaT_d = nc.declare_dram_parameter("UaT", [h, h], F32R, isOutput=False)
    waT_d = nc.declare_dram_parameter("WaT", [h, h], F32R, isOutput=False)
    # packed small consts: cols [0:kb_n]=VaT, [kb_n:2kb_n]=biasT,
    # [2kb_n : 2kb_n + ht*b_loc]=qT  (qTp[p, i*b_loc+b] = q[b, i*128+p])
    smallc_d = nc.declare_dram_parameter(
        "smallc", [128, 2 * kb_n + ht * b_loc], F32R, isOutput=False
    )
    ident_d = nc.declare_dram_parameter("ident", [128, 128], F32R, isOutput=False)
    ctx_d = nc.declare_dram_parameter("ctx", [b_loc, h], F32, isOutput=True)
    wts_d = nc.declare_dram_parameter("wts", [b_loc, s], F32, isOutput=True)

    with tile.TileContext(nc) as tc:
        with (
            tc.tile_pool(name="const", bufs=1) as const_pool,
            tc.tile_pool(name="keysp", bufs=20) as keys_pool,
        ):
            ident = const_pool.tile([128, 128], F32R, tag="ident")
            nc.scalar.dma_start(ident[:], ident_d[:])

            smallc = const_pool.tile([128, 2 * kb_n + ht * b_loc], F32R, tag="smallc")
            nc.scalar.dma_start(smallc[:], smallc_d[:])
            vaT_sb = smallc[:, 0:kb_n]
            biasT_sb = smallc[:, kb_n : 2 * kb_n]
            qT_sb = smallc[:, 2 * kb_n : 2 * kb_n + ht * b_loc]

            # per-(k, batch) bias for the tanh: q_proj[k] + Wa_b[k] + Ua_b[k],
            # laid out [128k, kb * b_loc + b]
            qbias_sb = const_pool.tile([128, kb_n * b_loc], F32, tag="qbias")
            den_sb = const_pool.tile([1, b_loc * n_ssb], F32, tag="den")
            dsum_sb = const_pool.tile([1, b_loc], F32, tag="dsum")
            inv_sb = const_pool.tile([1, b_loc], F32, tag="inv")

            # ---- preamble: q_projT[k, b] = WaT.T @ qT, + combined bias ----
            with (
                tc.tile_pool(name="psq", bufs=kb_n, space="PSUM") as psq,
                tc.tile_pool(name="watp", bufs=2) as watp,
            ):
                qp_tiles = []
                for kb in range(kb_n):
                    qp = psq.tile([128, b_loc], F32, name=f"qp{kb}", tag="qp")
                    qp_tiles.append(qp)
                for i in range(ht):
                    wat_t = watp.tile([128, h], F32R, name="wat_t", tag="wat")
                    nc.scalar.dma_start(wat_t[:], waT_d[i * 128 : (i + 1) * 128, :])
                    for kb in range(kb_n):
                        nc.tensor.matmul(
                            qp_tiles[kb][:],
                            wat_t[:, kb * 128 : (kb + 1) * 128],
                            qT_sb[:, i * b_loc : (i + 1) * b_loc],
                            start=(i == 0),
                            stop=(i == ht - 1),
                        )
                for kb in range(kb_n):
                    nc.scalar.activation(
                        qbias_sb[:, kb * b_loc : (kb + 1) * b_loc],
                        qp_tiles[kb][:],
                        mybir.ActivationFunctionType.Identity,
                        bias=biasT_sb[:, kb : kb + 1],
                        scale=1.0,
                    )

            # UaT loads go on the scalar queue AFTER WaT: the preamble needs
            # WaT immediately, the first k_proj matmul needs UaT ~20us in.
            uaT_sb = const_pool.tile([128, ht * h], F32R, tag="uaT")
            for i in range(ht):
                nc.scalar.dma_start(
                    uaT_sb[:, i * h : (i + 1) * h], uaT_d[i * 128 : (i + 1) * 128, :]
                )

            # ---- main pipeline ----
            with (
                tc.tile_pool(name="psum", bufs=2, space="PSUM") as psum_pool,
                tc.tile_pool(name="ktp", bufs=16) as kt_pool,
                tc.tile_pool(name="etp", bufs=3) as e_pool,
                tc.tile_pool(name="expp", bufs=2) as exp_pool,
                tc.tile_pool(name="misc", bufs=2) as misc_pool,
            ):
                work = [(b, sb) for b in range(b_loc) for sb in range(n_ssb)]
                keys_tiles = {}  # (b, s_tile) -> sbuf tile
                kt_sb = {}       # work idx -> list of ht transposed tiles
                exp_rows = {}    # b -> exp row tile

                def emit_transpose_stage(idx):
                    b, sb = work[idx]
                    for j in range(4):
                        st = sb * 4 + j
                        ktile = keys_pool.tile(
                            [128, h], F32R, name=f"k_{b}_{st}", tag="keys"
                        )
                        nc.sync.dma_start(
                            ktile[:], keys_d[b, st * 128 : (st + 1) * 128, :]
                        )
                        keys_tiles[(b, st)] = ktile
                    tiles = []
                    for i in range(ht):
                        ktr = psum_pool.tile([128, ssb], F32R, name="ktr", tag="ktr")
                        for j in range(4):
                            nc.tensor.transpose(
                                ktr[:, j * 128 : (j + 1) * 128],
                                keys_tiles[(b, sb * 4 + j)][:, i * 128 : (i + 1) * 128],
                                ident[:],
                            )
                        kts = kt_pool.tile([128, ssb], F32R, name="kts", tag="kts")
                        nc.vector.tensor_copy(kts[:], ktr[:])
                        tiles.append(kts)
                    kt_sb[idx] = tiles

                def emit_mm_stage(idx):
                    b, sb = work[idx]
                    if sb == 0:
                        exp_rows[b] = exp_pool.tile(
                            [1, s], F32R, name=f"exprow{b}", tag="exprow"
                        )
                    sc = psum_pool.tile([1, ssb], F32, name="sc", tag="scetr")
                    for kb in range(kb_n):
                        kp = psum_pool.tile([128, ssb], F32, name="kp", tag="kp")
                        for i in range(ht):
                            nc.tensor.matmul(
                                kp[:],
                                uaT_sb[:, i * h + kb * 128 : i * h + (kb + 1) * 128],
                                kt_sb[idx][i][:],
                                start=(i == 0),
                                stop=(i == ht - 1),
                            )
                        et = e_pool.tile([128, ssb], F32R, name="et", tag="et")
                        nc.scalar.activation(
                            et[:],
                            kp[:],
                            mybir.ActivationFunctionType.Tanh,
                            bias=qbias_sb[:, kb * b_loc + b : kb * b_loc + b + 1],
                            scale=1.0,
                        )
                        nc.tensor.matmul(
                            sc[:],
                            vaT_sb[:, kb : kb + 1],
                            et[:],
                            start=(kb == 0),
                            stop=(kb == kb_n - 1),
                        )
                    del kt_sb[idx]
                    nc.scalar.activation(
                        exp_rows[b][:, sb * ssb : (sb + 1) * ssb],
                        sc[:],
                        mybir.ActivationFunctionType.Exp,
                        accum_out=den_sb[:, b * n_ssb + sb : b * n_ssb + sb + 1],
                    )

                def emit_batch_tail(b):
                    nc.vector.tensor_reduce(
                        dsum_sb[:, b : b + 1],
                        den_sb[:, b * n_ssb : (b + 1) * n_ssb],
                        axis=mybir.AxisListType.X,
                        op=mybir.AluOpType.add,
                    )
                    nc.vector.reciprocal(inv_sb[:, b : b + 1], dsum_sb[:, b : b + 1])
                    wrow = misc_pool.tile([1, s], F32, name="wrow", tag="wrow", bufs=1)
                    nc.vector.tensor_scalar_mul(
                        wrow[:], exp_rows[b][:], inv_sb[:, b : b + 1]
                    )
                    nc.sync.dma_start(wts_d[b, :], wrow[:])
                    # transpose the exp row into [s, 1] columns for the context
                    # matmul. fp32r transposes need an even moving-dim count, so
                    # first reshape the row onto 4 partitions ([4, 512] via a
                    # SBUF->SBUF DMA), then run 4 transposes of [4, 128].
                    # Transpose j writes columns j*4..j*4+3 = s-tiles j, 4+j,
                    # 8+j, 12+j; consumers index with the matching permutation.
                    exp4 = misc_pool.tile(
                        [4, s // 4], F32R, name="exp4", tag="exp4", bufs=1
                    )
                    nc.sync.dma_start(exp4[:], exp_rows[b][:])
                    etr = psum_pool.tile([128, n_st], F32R, name="etr", tag="scetr")
                    for j in range(n_ssb):
                        nc.tensor.transpose(
                            etr[:, j * 4 : (j + 1) * 4],
                            exp4[:, j * 128 : (j + 1) * 128],
                            ident[0:4, 0:4],
                        )
                    etr_sb = misc_pool.tile([128, n_st], F32R, name="etr_sb", tag="etr_sb")
                    nc.vector.tensor_copy(etr_sb[:], etr[:])
                    del exp_rows[b]
                    cxp = [
                        psum_pool.tile([1, 512], F32, name=f"cxp{hh}", tag="cxp")
                        for hh in range(n_hh)
                    ]
                    for j in range(n_st):
                        ec = (j % n_ssb) * 4 + j // n_ssb  # etr column for s-tile j
                        for hh in range(n_hh):
                            nc.tensor.matmul(
                                cxp[hh][:],
                                etr_sb[:, ec : ec + 1],
                                keys_tiles[(b, j)][:, hh * 512 : (hh + 1) * 512],
                                start=(j == 0),
                                stop=(j == n_st - 1),
                            )
                    crow = misc_pool.tile([1, h], F32, name="crow", tag="crow", bufs=1)
                    for hh in range(n_hh):
                        nc.scalar.activation(
                            crow[:, hh * 512 : (hh + 1) * 512],
                            cxp[hh][:],
                            mybir.ActivationFunctionType.Copy,
                            bias=0.0,
                            scale=inv_sb[:, b : b + 1],
                        )
                    for st in range(n_st):
                        del keys_tiles[(b, st)]
                    nc.sync.dma_start(ctx_d[b, :], crow[:])

                # 1-deep software pipeline: transposes for idx run ahead of the
                # matmul stage of idx-1 so the PE never waits on the DVE copies.
                for idx in range(len(work) + 1):
                    if idx < len(work):
                        emit_transpose_stage(idx)
                    if idx > 0:
                        emit_mm_stage(idx - 1)
                        b, sb = work[idx - 1]
                        if sb == n_ssb - 1:
                            emit_batch_tail(b)

    nc.compile()
    return nc


def _shard_inputs(query, keys, Wa_w, Wa_b, Ua_w, Ua_b, Va_w, Va_b):
    query = np.asarray(query, dtype=np.float32)
    keys = np.asarray(keys, dtype=np.float32)
    uaT = np.ascontiguousarray(np.asarray(Ua_w, dtype=np.float32).T)
    waT = np.ascontiguousarray(np.asarray(Wa_w, dtype=np.float32).T)
    vaT = np.asarray(Va_w, dtype=np.float32)[0].reshape(H // 128, 128).T
    biasT = (
        np.asarray(Wa_b, dtype=np.float32) + np.asarray(Ua_b, dtype=np.float32)
    ).reshape(H // 128, 128).T
    ident = np.eye(128, dtype=np.float32)
    in_maps = []
    for c in range(N_CORES):
        sl = slice(c * B_LOC, (c + 1) * B_LOC)
        in_maps.append(
            {
                "keys": np.ascontiguousarray(keys[sl]),
                "UaT": uaT,
                "WaT": waT,
                # qTp[p, i*B_LOC+b] = query[b, i*128+p]
                "smallc": np.ascontiguousarray(
                    np.concatenate(
                        [
                            vaT,
                            biasT,
                            query[sl, 0, :]
                            .T.reshape(H // 128, 128, -1)
                            .transpose(1, 0, 2)
                            .reshape(128, -1),
                        ],
                        axis=1,
                    )
                ),
                "ident": ident,
            }
        )
    return in_maps


_NC_CACHE = {}


def run(trace=False, **inputs):
    if "nc" not in _NC_CACHE:
        _NC_CACHE["nc"] = build_bass()
    nc = _NC_CACHE["nc"]
    in_maps = _shard_inputs(**inputs)
    res = run_bass_kernel_spmd(nc, in_maps, list(range(N_CORES)), trace=trace)
    context = np.concatenate([res.results[c]["ctx"] for c in range(N_CORES)], axis=0)
    weights = np.concatenate([res.results[c]["wts"] for c in range(N_CORES)], axis=0)
    context = context.reshape(B, 1, H).astype(np.float32)
    weights = weights.reshape(B, 1, S).astype(np.float32)
    return (context, weights), res


def kernel(**inputs):
    (context, weights), _ = run(trace=False, **inputs)
    return (context, weights)


# revision 24
# speedup vs baseline: 1.0102x; 1.0102x over previous
"""Bahdanau attention on Trainium2, data-parallel over batch across 8 NeuronCores.

reference math (per batch b):
  q_proj = query @ Wa_w.T + Wa_b            # [1, H]
  k_proj = keys  @ Ua_w.T + Ua_b            # [S, H]
  e      = tanh(q_proj + k_proj)            # [S, H]
  scores = e @ Va_w[0] + Va_b[0]            # [S]
  weights= softmax(scores)                  # [S]
  context= weights @ keys                   # [H]

Device strategy (per core, B_loc = 4 batches):
  - keys tiles [128s, 1024h] stay resident in SBUF for the whole batch; they
    serve both the k_proj matmul (after a PE transpose to [h, s] layout) and
    the context matmul (natural layout, contraction over s).
  - k_proj is computed transposed: kprojT[k, s] = UaT.T @ keysT, with UaT
    (the [h, k]-layout Ua) resident in SBUF; fp32r matmuls at full PE rate.
  - tanh folds in the per-k bias (q_proj[k] + Wa_b[k] + Ua_b[k]) via the
    ACT per-partition bias operand; Va dot-product is a PE matvec over k.
  - softmax skips the max-subtraction (|scores| <= ||Va||_1 ~ 16, exp is
    safe in fp32; softmax is shift-invariant so Va_b drops out entirely).
    The denominator comes free from the ACT accum_out of the exp.
"""

import sys

import numpy as np

try:
    import concourse.tile as tile
    from concourse import bacc, mybir
    from concourse.bass_utils import run_bass_kernel_spmd
except ImportError:
    for _p in ("/opt/trn_rl_repo", "/root/.axon_site/_ro/trn_rl_repo"):
        if _p not in sys.path:
            sys.path.insert(0, _p)
    import concourse.tile as tile
    from concourse import bacc, mybir
    from concourse.bass_utils import run_bass_kernel_spmd

F32 = mybir.dt.float32
F32R = mybir.dt.float32r

B, S, H = 32, 2048, 1024
N_CORES = 8
B_LOC = B // N_CORES  # 4


def build_bass(b_loc=B_LOC, s=S, h=H):
    ht = h // 128    # h-tiles (contraction tiles)
    kb_n = h // 128  # k-blocks (output tiles of k_proj)
    ssb = 512        # s-superblock: matmul moving-dim size
    n_ssb = s // ssb
    n_st = s // 128  # s-tiles per batch
    n_hh = h // 512  # 512-wide column halves for the context matmul

    nc = bacc.Bacc("TRN2", target_bir_lowering=False, debug=False)

    keys_d = nc.declare_dram_parameter("keys", [b_loc, s, h], F32R, isOutput=False)
    uaT_d = nc.declare_dram_parameter("UaT", [h, h], F32R, isOutput=False)
    waT_d = nc.declare_dram_parameter("WaT", [h, h], F32R, isOutput=False)
    # packed small consts: cols [0:kb_n]=VaT, [kb_n:2kb_n]=biasT,
    # [2kb_n : 2kb_n + ht*b_loc]=qT  (qTp[p, i*b_loc+b] = q[b, i*128+p])
    smallc_d = nc.declare_dram_parameter(
        "smallc", [128, 2 * kb_n + ht * b_loc], F32R, isOutput=False
    )
    ident_d = nc.declare_dram_parameter("ident", [128, 128], F32R, isOutput=False)
    ctx_d = nc.declare_dram_parameter("ctx", [b_loc, h], F32, isOutput=True)
    wts_d = nc.declare_dram_parameter("wts", [b_loc, s], F32, isOutput=True)

    with tile.TileContext(nc) as tc:
        with (
            tc.tile_pool(name="const", bufs=1) as const_pool,
            tc.tile_pool(name="keysp", bufs=21) as keys_pool,
        ):
            smallc = const_pool.tile([128, 2 * kb_n + ht * b_loc], F32R, tag="smallc")
            nc.scalar.dma_start(smallc[:], smallc_d[:])
            ident = const_pool.tile([128, 128], F32R, tag="ident")
            vaT_sb = smallc[:, 0:kb_n]
            biasT_sb = smallc[:, kb_n : 2 * kb_n]
            qT_sb = smallc[:, 2 * kb_n : 2 * kb_n + ht * b_loc]

            # per-(k, batch) bias for the tanh: q_proj[k] + Wa_b[k] + Ua_b[k],
            # laid out [128k, kb * b_loc + b]
            qbias_sb = const_pool.tile([128, kb_n * b_loc], F32, tag="qbias")
            den_sb = const_pool.tile([1, b_loc * n_ssb], F32, tag="den")
            dsum_sb = const_pool.tile([1, b_loc], F32, tag="dsum")
            inv_sb = const_pool.tile([1, b_loc], F32, tag="inv")

            # ---- preamble: q_projT[k, b] = WaT.T @ qT, + combined bias ----
            with (
                tc.tile_pool(name="psq", bufs=kb_n, space="PSUM") as psq,
                tc.tile_pool(name="watp", bufs=2) as watp,
            ):
                qp_tiles = []
                for kb in range(kb_n):
                    qp = psq.tile([128, b_loc], F32, name=f"qp{kb}", tag="qp")
                    qp_tiles.append(qp)
                for i in range(ht):
                    wat_t = watp.tile([128, h], F32R, name="wat_t", tag="wat")
                    nc.scalar.dma_start(wat_t[:], waT_d[i * 128 : (i + 1) * 128, :])
                    for kb in range(kb_n):
                        nc.tensor.matmul(
                            qp_tiles[kb][:],
                            wat_t[:, kb * 128 : (kb + 1) * 128],
                            qT_sb[:, i * b_loc : (i + 1) * b_loc],
                            start=(i == 0),
                            stop=(i == ht - 1),
                        )
                for kb in range(kb_n):
                    nc.scalar.activation(
                        qbias_sb[:, kb * b_loc : (kb + 1) * b_loc],
                        qp_tiles[kb][:],
                        mybir.ActivationFunctionType.Identity,
                        bias=biasT_sb[:, kb : kb + 1],
                        scale=1.0,
                    )

            # Startup is DMA-bandwidth-bound: ~12MB (WaT + UaT + first keys)
            # must land before the first k_proj group. Order by first use:
            # scalar queue: smallc, WaT (in preamble), ident, UaT[hi];
            # sync queue: keys[ssb0], UaT[lo], keys[ssb1], ...
            nc.scalar.dma_start(ident[:], ident_d[:])
            uaT_sb = const_pool.tile([128, ht * h], F32R, tag="uaT")

            def emit_uaT_loads(lo_hi):
                rng = range(ht // 2) if lo_hi == 0 else range(ht // 2, ht)
                eng = nc.sync if lo_hi == 0 else nc.scalar
                for i in rng:
                    eng.dma_start(
                        uaT_sb[:, i * h : (i + 1) * h],
                        uaT_d[i * 128 : (i + 1) * 128, :],
                    )

            emit_uaT_loads(1)

            # ---- main pipeline ----
            with (
                tc.tile_pool(name="psum", bufs=2, space="PSUM") as psum_pool,
                tc.tile_pool(name="ktp", bufs=16) as kt_pool,
                tc.tile_pool(name="etp", bufs=3) as e_pool,
                tc.tile_pool(name="expp", bufs=2) as exp_pool,
                tc.tile_pool(name="misc", bufs=2) as misc_pool,
            ):
                work = [(b, sb) for b in range(b_loc) for sb in range(n_ssb)]
                keys_tiles = {}  # (b, s_tile) -> sbuf tile
                kt_sb = {}       # work idx -> list of ht transposed tiles
                exp_rows = {}    # b -> exp row tile

                def emit_transpose_stage(idx):
                    b, sb = work[idx]
                    for j in range(4):
                        st = sb * 4 + j
                        ktile = keys_pool.tile(
                            [128, h], F32R, name=f"k_{b}_{st}", tag="keys"
                        )
                        nc.sync.dma_start(
                            ktile[:], keys_d[b, st * 128 : (st + 1) * 128, :]
                        )
                        keys_tiles[(b, st)] = ktile
                    tiles = []
                    for i in range(ht):
                        ktr = psum_pool.tile([128, ssb], F32R, name="ktr", tag="ktr")
                        for j in range(4):
                            nc.tensor.transpose(
                                ktr[:, j * 128 : (j + 1) * 128],
                                keys_tiles[(b, sb * 4 + j)][:, i * 128 : (i + 1) * 128],
                                ident[:],
                            )
                        kts = kt_pool.tile([128, ssb], F32R, name="kts", tag="kts")
                        nc.vector.tensor_copy(kts[:], ktr[:])
                        tiles.append(kts)
                    kt_sb[idx] = tiles

                def emit_mm_stage(idx):
                    b, sb = work[idx]
                    if sb == 0:
                        exp_rows[b] = exp_pool.tile(
                            [1, s], F32R, name=f"exprow{b}", tag="exprow"
                        )
                    sc = psum_pool.tile([1, ssb], F32, name="sc", tag="scetr")
                    for kb in range(kb_n):
                        kp = psum_pool.tile([128, ssb], F32, name="kp", tag="kp")
                        for i in range(ht):
                            nc.tensor.matmul(
                                kp[:],
                                uaT_sb[:, i * h + kb * 128 : i * h + (kb + 1) * 128],
                                kt_sb[idx][i][:],
                                start=(i == 0),
                                stop=(i == ht - 1),
                            )
                        et = e_pool.tile([128, ssb], F32R, name="et", tag="et")
                        nc.scalar.activation(
                            et[:],
                            kp[:],
                            mybir.ActivationFunctionType.Tanh,
                            bias=qbias_sb[:, kb * b_loc + b : kb * b_loc + b + 1],
                            scale=1.0,
                        )
                        nc.tensor.matmul(
                            sc[:],
                            vaT_sb[:, kb : kb + 1],
                            et[:],
                            start=(kb == 0),
                            stop=(kb == kb_n - 1),
                        )
                    del kt_sb[idx]
                    nc.scalar.activation(
                        exp_rows[b][:, sb * ssb : (sb + 1) * ssb],
                        sc[:],
                        mybir.ActivationFunctionType.Exp,
                        accum_out=den_sb[:, b * n_ssb + sb : b * n_ssb + sb + 1],
                    )

                def emit_batch_tail(b):
                    nc.vector.tensor_reduce(
                        dsum_sb[:, b : b + 1],
                        den_sb[:, b * n_ssb : (b + 1) * n_ssb],
                        axis=mybir.AxisListType.X,
                        op=mybir.AluOpType.add,
                    )
                    nc.vector.reciprocal(inv_sb[:, b : b + 1], dsum_sb[:, b : b + 1])
                    wrow = misc_pool.tile([1, s], F32, name="wrow", tag="wrow", bufs=1)
                    nc.vector.tensor_scalar_mul(
                        wrow[:], exp_rows[b][:], inv_sb[:, b : b + 1]
                    )
                    nc.sync.dma_start(wts_d[b, :], wrow[:])
                    # transpose the exp row into [s, 1] columns for the context
                    # matmul. fp32r transposes need an even moving-dim count, so
                    # first reshape the row onto 4 partitions ([4, 512] via a
                    # SBUF->SBUF DMA), then run 4 transposes of [4, 128].
                    # Transpose j writes columns j*4..j*4+3 = s-tiles j, 4+j,
                    # 8+j, 12+j; consumers index with the matching permutation.
                    exp4 = misc_pool.tile(
                        [4, s // 4], F32R, name="exp4", tag="exp4", bufs=1
                    )
                    nc.sync.dma_start(exp4[:], exp_rows[b][:])
                    etr = psum_pool.tile([128, n_st], F32R, name="etr", tag="scetr")
                    for j in range(n_ssb):
                        nc.tensor.transpose(
                            etr[:, j * 4 : (j + 1) * 4],
                            exp4[:, j * 128 : (j + 1) * 128],
                            ident[0:4, 0:4],
                        )
                    etr_sb = misc_pool.tile([128, n_st], F32R, name="etr_sb", tag="etr_sb")
                    nc.vector.tensor_copy(etr_sb[:], etr[:])
                    del exp_rows[b]
                    cxp = [
                        psum_pool.tile([1, 512], F32, name=f"cxp{hh}", tag="cxp")
                        for hh in range(n_hh)
                    ]
                    for j in range(n_st):
                        ec = (j % n_ssb) * 4 + j // n_ssb  # etr column for s-tile j
                        for hh in range(n_hh):
                            nc.tensor.matmul(
                                cxp[hh][:],
                                etr_sb[:, ec : ec + 1],
                                keys_tiles[(b, j)][:, hh * 512 : (hh + 1) * 512],
                                start=(j == 0),
                                stop=(j == n_st - 1),
                            )
                    crow = misc_pool.tile([1, h], F32, name="crow", tag="crow", bufs=1)
                    for hh in range(n_hh):
                        nc.scalar.activation(
                            crow[:, hh * 512 : (hh + 1) * 512],
                            cxp[hh][:],
                            mybir.ActivationFunctionType.Copy,
                            bias=0.0,
                            scale=inv_sb[:, b : b + 1],
                        )
                    for st in range(n_st):
                        del keys_tiles[(b, st)]
                    nc.sync.dma_start(ctx_d[b, :], crow[:])

                # 1-deep software pipeline: transposes for idx run ahead of the
                # matmul stage of idx-1 so the PE never waits on the DVE copies.
                for idx in range(len(work) + 1):
                    if idx < len(work):
                        emit_transpose_stage(idx)
                    if idx == 0:
                        emit_uaT_loads(0)
                    if idx > 0:
                        emit_mm_stage(idx - 1)
                        b, sb = work[idx - 1]
                        if sb == n_ssb - 1:
                            emit_batch_tail(b)

    nc.compile()
    return nc


def _shard_inputs(query, keys, Wa_w, Wa_b, Ua_w, Ua_b, Va_w, Va_b):
    query = np.asarray(query, dtype=np.float32)
    keys = np.asarray(keys, dtype=np.float32)
    uaT = np.ascontiguousarray(np.asarray(Ua_w, dtype=np.float32).T)
    waT = np.ascontiguousarray(np.asarray(Wa_w, dtype=np.float32).T)
    vaT = np.asarray(Va_w, dtype=np.float32)[0].reshape(H // 128, 128).T
    biasT = (
        np.asarray(Wa_b, dtype=np.float32) + np.asarray(Ua_b, dtype=np.float32)
    ).reshape(H // 128, 128).T
    ident = np.eye(128, dtype=np.float32)
    in_maps = []
    for c in range(N_CORES):
        sl = slice(c * B_LOC, (c + 1) * B_LOC)
        in_maps.append(
            {
                "keys": np.ascontiguousarray(keys[sl]),
                "UaT": uaT,
                "WaT": waT,
                # qTp[p, i*B_LOC+b] = query[b, i*128+p]
                "smallc": np.ascontiguousarray(
                    np.concatenate(
                        [
                            vaT,
                            biasT,
                            query[sl, 0, :]
                            .T.reshape(H // 128, 128, -1)
                            .transpose(1, 0, 2)
                            .reshape(128, -1),
                        ],
                        axis=1,
                    )
                ),
                "ident": ident,
            }
        )
    return in_maps


_NC_CACHE = {}


def run(trace=False, **inputs):
    if "nc" not in _NC_CACHE:
        _NC_CACHE["nc"] = build_bass()
    nc = _NC_CACHE["nc"]
    in_maps = _shard_inputs(**inputs)
    res = run_bass_kernel_spmd(nc, in_maps, list(range(N_CORES)), trace=trace)
    context = np.concatenate([res.results[c]["ctx"] for c in range(N_CORES)], axis=0)
    weights = np.concatenate([res.results[c]["wts"] for c in range(N_CORES)], axis=0)
    context = context.reshape(B, 1, H).astype(np.float32)
    weights = weights.reshape(B, 1, S).astype(np.float32)
    return (context, weights), res


def kernel(**inputs):
    (context, weights), _ = run(trace=False, **inputs)
    return (context, weights)
